# revision 90
# baseline (speedup 1.0000x reference)
"""Trainium2 Bass kernel for nn_Block_35880156790920 (dense transformer block).

Sharding: 8 cores = 2 batches x 4 query-token-blocks; K/V for the other
3 blocks of each batch arrive via an AllGather within each 4-core group.

Per-core pipeline (bf16 matmul operands, fp32 accumulate):
  LN1 (token-major, fp32 stats) -> batched DMA-transpose -> xnqT
  projections in order K, V, Q so the K and V AllGather + scatter chains
    hide behind the remaining projections (V's eviction is cheap; Q's
    DVE-bound l2norm chain overlaps the V collective)
  l2norm reads the projection PSUM directly; biases via K=1 matmuls
  per-group contiguous scatters into group-major knT / token-major vaug
  attention per head pair: scoresT -> exp (ACT, 2 heads/op) -> ctxT +
    softmax denominator via [v|1] matmul accumulation; per-pair bf16
    denominator normalization deferred one pair to hide DMA latency
  out-proj + residual, LN2, MLP with w1/w2 streamed in gated pieces
    (dead-region reuse: wv/wo/h1gA share a slot, w1 head in the w slot,
    w2 in knT/vaug), residual -> y.

All heavy weight loads ride the gpsimd/SWDGE or ACT DGE rings in 512KB
pieces so latency-critical small DMAs never queue behind them.
"""

from contextlib import ExitStack

import numpy as np
import ml_dtypes

import concourse.bass as bass
import concourse.tile as tile
from concourse import bacc, mybir
from concourse.bass import ts, ds
from concourse.bass_utils import run_bass_kernel_spmd

F32 = mybir.dt.float32
F32R = mybir.dt.float32r
BF16 = mybir.dt.bfloat16
F8 = mybir.dt.float8e4
AF = mybir.ActivationFunctionType
ALU = mybir.AluOpType

P = 128
B, S, D = 2, 2048, 1024
H, HD = 16, 64
MLP = 4096
SQ = S // 4          # 512 query tokens per core
DC = D // P          # 8
TB = S // P          # 16
TQ = SQ // P         # 4
MC = MLP // P        # 32
HP = H // 2          # 8 head pairs
EPS_LN = 1e-6
EPS_NORM = 1e-12
LOG_MAX = float(np.log(1.0 / 0.01))
N_CORES = 8
SKIP_CC = False

_CACHED_NC = {}


def _emit_once(tc, outs, ins, pools):
    nc = tc.nc

    xq, xqr = ins["xq"], ins["xqr"]
    y = outs["y"]

    # ---- all own x tiles stream in one DMA before anything else so the
    # weight loads can't starve the LN1 critical path ----
    x_all = pools["xin"].tile([P, TQ, D], F32, tag="x", name="x")
    nc.sync.dma_start(x_all[:, 0:1, :],
                      xq[0:P, :].rearrange("(t p) d -> p t d", p=P))
    nc.sync.dma_start(x_all[:, 1:TQ, :],
                      xq[P:SQ, :].rearrange("(t p) d -> p t d", p=P))

    # ---- constants ----
    eps_tile = pools["const"].tile([P, 1], F32, tag="eps", name="eps")
    nc.vector.memset(eps_tile[:], EPS_LN)
    eps0 = pools["const"].tile([P, 1], F32, tag="eps0", name="eps0")
    nc.vector.memset(eps0[:], 0.0)
    ones_tok = pools["const"].tile([1, P], BF16, tag="ones_tok", name="ones_tok")
    nc.vector.memset(ones_tok[:], 1.0)
    ones_hd = pools["const"].tile([1, HD], BF16, tag="ones_hd", name="ones_hd")
    nc.vector.memset(ones_hd[:], 1.0)

    # b2 rows streamed per-use into [1,512] scratch tiles (fc2 only)
    def bias_rhs(name, n):
        rrow = pools["rrow"].tile([1, 512], BF16, tag="rrow", name="rrow")
        nc.sync.dma_start(rrow[:], ins[name][0:1, ts(n, 512)])
        return rrow[:]

    bias_m = pools["const"].tile([P, MC], F32, tag="bias_m", name="bias_m")
    nc.scalar.dma_start(bias_m[:], ins["bias_m"][:])

    # touch every ACT function once so the table loads happen at t=0, off the
    # LN1 critical path
    actwarm = pools["const"].tile([1, 4], F32, tag="actwarm", name="actwarm")
    for fn in (AF.Sqrt, AF.Square, AF.Exp, AF.Gelu):
        nc.scalar.activation(actwarm[0:1, 0:1], eps_tile[0:1, 0:1], fn)

    # per-head scale c = exp(min(logit_scale, LOG_MAX)), broadcast on partitions
    crow = pools["const"].tile([1, H], F32, tag="crow", name="crow")
    nc.sync.dma_start(crow[:], ins["ck"][:])
    c_b = pools["const"].tile([P, H], F32, tag="c_b", name="c_b")
    nc.gpsimd.partition_broadcast(c_b[:], crow[:])

    # ---- persistent activations ----
    xnqT = pools["xnqT"].tile([P, DC, SQ], BF16, tag="xnqT", name="xnqT")   # 1 MB
    # group-major K layout: [P, gather-group, head-pair, 512 keys] so each
    # gather group scatters as one fully-contiguous DMA
    knT = pools["knT"].tile([P, 4, DC, S // 4], BF16, tag="knT", name="knT")  # 4 MB
    qnT = pools["qnT"].tile([P, DC, SQ], BF16, tag="qnT", name="qnT")      # 1 MB
    vaug = pools["vaug"].tile([P, TB, H, HD + 1], BF16, tag="vaug", name="vaug")  # 4.25 MB
    ao_dram = pools["dram"].tile([SQ, D], F32, tag="aodram", name="aodram")
    knTo = pools["ctxU"].tile([P, DC, SQ], BF16, tag="ctxU", name="knTo")
    vaugo = pools["ctxU"].tile([P, TQ, H, HD + 1], BF16, tag="btmp", name="vaugo")
    # per-head-pair softmax denominators: fresh [2, SQ] tile per pair (ring
    # of 2 — written at hp, consumed by the deferred normalize at hp+1)
    den_tiles = {}

    def den_tile(hp):
        if hp not in den_tiles:
            den_tiles[hp] = pools["den"].tile([2, SQ], BF16, tag="den",
                                              name="den")
        return den_tiles[hp]

    def den_row(h):
        return den_tile(h // 2)[h % 2:h % 2 + 1, :]

    # ones columns of own v-augmented (v evictions later overwrite cols 0:HD)
    nc.vector.memset(vaugo[:], 1.0)

    def ln_tile(x_ap, out_bf16_ap):
        """LayerNorm stats+apply for one [P, D] fp32 tile -> bf16 (gain folded
        into weights on host, ln-bias folded into projection bias rows)."""
        st = pools["stats"].tile([P, 2, 6], F32, tag="st", name="st")
        xr = x_ap.rearrange("p (s d) -> p s d", s=2)
        for i in range(2):
            nc.vector.bn_stats(st[:, i, :], xr[:, i, :])
        mv = pools["stats"].tile([P, 2], F32, tag="mv", name="mv")
        nc.vector.bn_aggr(mv[:], st[:])
        rstd = pools["stats"].tile([P, 1], F32, tag="rstd", name="rstd")
        nc.scalar.activation(rstd[:], mv[:, 1:2], AF.Sqrt, bias=eps_tile[:])
        nc.vector.reciprocal(rstd[:], rstd[:])
        nc.vector.tensor_scalar(out_bf16_ap, x_ap, scalar1=mv[:, 0:1],
                                scalar2=rstd[:], op0=ALU.subtract, op1=ALU.mult)

    # ---- LN1 over own tokens -> xnqT (earliest priority band: nothing may
    # get scheduled ahead of this chain on DVE/ACT/SP) ----
    xn_t0 = None
    with tc.high_priority():
        for t in range(TQ):
            xn_t = pools["xn"].tile([P, D], BF16, tag="xn", name="xn")
            if t == 0:
                xn_t0 = xn_t
            ln_tile(x_all[:, t, :], xn_t[:])
            # batched transpose: dst[p, d, tok] = src[tok, d*128+p]
            nc.sync.dma_start(xnqT[:, :, ts(t, P)], xn_t[:], transpose=True)

    # q/k/v projection bias rows, preloaded once (used as K=1 matmul rhs)
    bias_qkv = pools["const"].tile([1, 3, D], BF16, tag="bias_qkv",
                                   name="bias_qkv")
    nc.sync.dma_start(bias_qkv[:], ins["bias_qkv"][:])

    # ---- QKV projections ----
    def l2norm_scale_transpose(t, kq_t, dstT, scale_pp):
        """kq_t: [P, D] token-major (psum fp32 ok); optional scale_pp [P, H]
        extra multiplier; writes l2-normalized transpose to dstT[..., t]."""
        sq = pools["eT"].tile([P, D], BF16, tag="eT", name="sq")
        nc.scalar.activation(sq[:], kq_t, AF.Square)
        ss = pools["stats"].tile([P, H], F32, tag="ss", name="ss")
        nc.vector.tensor_reduce(ss[:], sq[:].rearrange("p (h d) -> p h d", h=H),
                                axis=mybir.AxisListType.X, op=ALU.add)
        nrm = pools["stats"].tile([P, H], F32, tag="nrm", name="nrm")
        nc.scalar.activation(nrm[:], ss[:], AF.Sqrt, bias=eps0[:])
        nc.vector.tensor_scalar_max(nrm[:], nrm[:], EPS_NORM)
        rinv = pools["stats"].tile([P, H], F32, tag="rinv", name="rinv")
        nc.vector.reciprocal(rinv[:], nrm[:])
        if scale_pp is not None:
            nc.vector.tensor_tensor(rinv[:], rinv[:], scale_pp, op=ALU.mult)
        kn_t = pools["xn"].tile([P, D], BF16, tag="xn", name="xn")
        nc.vector.tensor_tensor(
            kn_t[:].rearrange("p (h d) -> p h d", h=H),
            kq_t.rearrange("p (h d) -> p h d", h=H),
            rinv[:, :, None].broadcast_to([P, H, HD]), op=ALU.mult)
        nc.sync.dma_start(dstT[:, :, ts(t, P)], kn_t[:], transpose=True)

    def evict_q(t, ps):
        l2norm_scale_transpose(t, ps, qnT, c_b[:])

    def evict_k(t, ps):
        l2norm_scale_transpose(t, ps, knTo, None)

    def evict_v(t, ps):
        nc.vector.tensor_copy(vaugo[:, t, :, 0:HD],
                              ps.rearrange("p (h d) -> p h d", h=H))

    def w_tile(slot):
        # double-buffer projection weights at zero SBUF cost: the B slot
        # borrows the h1gA region (dead until fc1)
        if slot == 0:
            return pools["w"].tile([P, DC, D], BF16, tag="w", name="w")
        return pools["xnT"].tile([P, DC, D], BF16, tag="xnTa", name="wB")

    def w_load(w_name, slot):
        # 512KB pieces so critical small DMAs can slot in between
        w_sb = w_tile(slot)
        for i in range(4):
            nc.gpsimd.dma_start(
                w_sb[:, :, ts(i, 256)],
                ins[w_name][:, ts(i, 256)].rearrange("(dc p) c -> p dc c", p=P))
        return w_sb

    def proj(w_sb, src_T, ntiles, evict, bias_idx):
        for t in range(ntiles):
            ps = pools["score"].tile([P, 1024], F32, tag="score", name="psqkv")
            for d in range(DC):
                lhs = src_T[:, d, ts(t, P)]
                nc.tensor.matmul(ps[:, 0:512], lhs, w_sb[:, d, 0:512],
                                 start=(d == 0), stop=False,
                                 skip_group_check=True)
                nc.tensor.matmul(ps[:, 512:1024], lhs, w_sb[:, d, 512:1024],
                                 start=(d == 0), stop=False,
                                 skip_group_check=True)
            for n in range(2):
                nc.tensor.matmul(ps[:, ts(n, 512)], ones_tok[:],
                                 bias_qkv[0:1, bias_idx, ts(n, 512)],
                                 start=False, stop=True, skip_group_check=True)
            evict(t, ps[:])

    KVK = DC * SQ
    KVV = TQ * H * (HD + 1)
    GROUPS = [[0, 1, 2, 3], [4, 5, 6, 7]]

    # K projection, then its gather + scatter run while V/Q projections run.
    # Scatters go on the gpsimd queue so they chase the collective FIFO-style
    # without blocking (or being blocked by) the SP/ACT DMA rings.
    # Projection order K, V, Q so the K gather/scatter chain (needed first by
    # attention) starts earliest. All projection weights load on the
    # otherwise-idle pool/SWDGE ring; scatters are fully contiguous per
    # partition so they cost ~1us each.
    # pool-queue gate: holds the weight pieces off the DMA engines until
    # LN1's first tile is through, protecting the LN1 -> K-proj chain
    poolgate = pools["dram"].tile([1, 2], BF16, tag="poolgate", name="poolgate")
    nc.gpsimd.dma_start(poolgate[:], xn_t0[0:1, 0:2])
    wk_sb = w_load("wk", 0)
    wq_sb = w_load("wq", 1)

    proj(wk_sb, xnqT, TQ, evict_k, 1)
    kb = pools["dram"].tile([P, KVK], F8, tag="kb", name="kb")
    kg = pools["dram"].tile([4, P, KVK], F8, tag="kg", name="kg")
    if True:
        nc.gpsimd.dma_start(kb[:], knTo[:].rearrange("p d s -> p (d s)"))
        if SKIP_CC:
            # timing stand-in for the collective: scatter straight from kb
            for g in range(4):
                nc.gpsimd.dma_start(
                    knT[:, g].rearrange("p d s -> p (d s)"), kb[:])
        else:
            nc.gpsimd.collective_compute(
                "AllGather", ALU.bypass, replica_groups=GROUPS,
                ins=[kb[:].opt()], outs=[kg[:].opt()])
            for g in range(4):
                nc.gpsimd.dma_start(
                    knT[:, g].rearrange("p d s -> p (d s)"), kg[g])
    wv_sb = w_load("wv", 0)  # WAR on K's matmuls — resolved by now

    # V before Q on hardware: the V AllGather needs the whole Q projection
    # to hide behind; Q's eviction chain (DVE) overlaps V's collective
    vb = pools["dram"].tile([P, KVV], F8, tag="vb", name="vb")
    vg = pools["dram"].tile([4, P, KVV], F8, tag="vg", name="vg")
    vbr = vb[:].rearrange("p (t h d) -> p t h d", t=TQ, h=H)

    def evict_v_spill(t, ps):
        evict_v(t, ps)
        # per-tile fp8 spill (SWDGE casts) so the V collective starts early
        # and moves half the bytes
        nc.gpsimd.dma_start(vbr[:, t], vaugo[:, t])

    proj(wv_sb, xnqT, TQ, evict_v_spill, 2)
    wo_sb = w_load("wo", 1)  # WAR on V's matmuls — resolved by now
    if SKIP_CC:
        for g in range(4):
            nc.gpsimd.dma_start(
                vaug[:, ds(TQ * g, TQ), :, :].rearrange("p t h d -> p (t h d)"),
                vb[:])
    else:
        nc.gpsimd.collective_compute(
            "AllGather", ALU.bypass, replica_groups=GROUPS,
            ins=[vb[:].opt()], outs=[vg[:].opt()])
        for g in range(4):
            nc.gpsimd.dma_start(
                vaug[:, ds(TQ * g, TQ), :, :].rearrange("p t h d -> p (t h d)"),
                vg[g])

    # q projection last: runs while the V gather/scatter chain is in flight
    proj(wq_sb, xnqT, TQ, evict_q, 0)

    # ---- attention: head pairs ----
    ctxU = pools["ctxU"].tile([P, DC, SQ], BF16, tag="ctxU", name="ctxU")
    btmp = pools["ctxU"].tile([HD, HP, SQ], BF16, tag="btmp", name="btmp")

    def emit_normalize(hp):
        # deferred one head pair so the rd0 chain latency is hidden behind
        # the next pair's score/exp/ctx stream
        for h in (2 * hp, 2 * hp + 1):
            rd0 = pools["rrow"].tile([1, SQ], BF16, tag="rrow", name="rd0")
            nc.sync.dma_start(rd0[:], den_row(h))
            dn = pools["mm512"].tile([P, 512], F32, tag="mm512", name="dn")
            nc.tensor.matmul(dn[0:HD, :], ones_hd[:], rd0[:],
                             start=True, stop=True)
            if h % 2 == 0:
                nc.vector.tensor_tensor(ctxU[0:HD, hp, :], ctxU[0:HD, hp, :],
                                        dn[0:HD, :], op=ALU.mult)
            else:
                nc.vector.tensor_tensor(btmp[:, hp, :], btmp[:, hp, :],
                                        dn[0:HD, :], op=ALU.mult)
                nc.sync.dma_start(ctxU[HD:P, hp, :], btmp[:, hp, :])

    for hp in range(HP):
        hA, hB = 2 * hp, 2 * hp + 1
        # alternate psum pools so the next pair's accumulators don't wait on
        # this pair's evictions (mm512 banks are idle during the hp loop)
        cpool = pools["ctx"] if hp % 2 == 0 else pools["mm512"]
        ctag = "ctx" if hp % 2 == 0 else "mm512"
        ctxA = cpool.tile([HD + 1, 512], F32, tag=ctag, name="ctx")
        ctxB = cpool.tile([HD + 1, 512], F32, tag=ctag, name="ctx")
        def emit_scores(kt):
            g, j = kt // 4, kt % 4
            sc = pools["score"].tile([P, 1024], F32, tag="score", name="score")
            nc.tensor.matmul(sc[:, 0:512], knT[0:HD, g, hp, ts(j, P)],
                             qnT[0:HD, hp, :], start=True, stop=True,
                             tile_position=(0, 0), skip_group_check=True)
            nc.tensor.matmul(sc[:, 512:1024], knT[HD:P, g, hp, ts(j, P)],
                             qnT[HD:P, hp, :], start=True, stop=True,
                             tile_position=(64, 0), skip_group_check=True)
            return sc

        # software pipeline: kt+1's scores issue on the PE before kt's ctx
        # matmuls, so the in-order PE never stalls waiting for exp(kt)
        sc = emit_scores(0)
        for kt in range(TB):
            eT = pools["eT"].tile([P, 1024], BF16, tag="eT", name="eT")
            nc.scalar.activation(eT[:], sc[:], AF.Exp)
            if kt + 1 < TB:
                sc = emit_scores(kt + 1)
            nc.tensor.matmul(ctxA[:], vaug[:, kt, hA, :], eT[:, 0:512],
                             start=(kt == 0), stop=(kt == TB - 1),
                             skip_group_check=True)
            nc.tensor.matmul(ctxB[:], vaug[:, kt, hB, :], eT[:, 512:1024],
                             start=(kt == 0), stop=(kt == TB - 1),
                             skip_group_check=True)
            if kt == 1 and hp >= 1:
                emit_normalize(hp - 1)
        # unnormalized evictions + denominator collection
        nc.vector.tensor_copy(ctxU[0:HD, hp, :], ctxA[0:HD, :])
        nc.vector.tensor_copy(btmp[:, hp, :], ctxB[0:HD, :])
        dtmp = pools["ao"].tile([HD + 1, 2, 512], BF16, tag="ao", name="dtmp")
        nc.vector.tensor_copy(dtmp[HD:HD + 1, 0, :], ctxA[HD:HD + 1, :])
        nc.vector.tensor_copy(dtmp[HD:HD + 1, 1, :], ctxB[HD:HD + 1, :])
        nc.sync.dma_start(den_row(hA), dtmp[HD:HD + 1, 0, :])
        nc.sync.dma_start(den_row(hB), dtmp[HD:HD + 1, 1, :])
        with nc.allow_low_precision(reason="softmax denom scale in bf16"):
            nc.vector.reciprocal(den_tile(hp)[:], den_tile(hp)[:])

    # ---- w1 prefetch: the first four m-tiles park in the qnT region (dead
    # after attention; its WAR releases the load at exactly the right time),
    # later chunks double-buffer through the w1 pool ----
    CH = 2                # m-tiles per w1 chunk
    NCH = MC // CH        # 16 chunks

    def w1_load(c):
        w1c = pools["w1"].tile([P, DC, CH * P], BF16, tag="w1", name="w1c")
        nc.scalar.dma_start(
            w1c[:],
            ins["w1"][:, ts(c, CH * P)].rearrange("(dc p) c -> p dc c", p=P))
        return w1c

    # first 8 m-tiles of w1 park in the w-pool slot, whose WAR releases at
    # V-proj end (~attention start, when the DMA engines are quiet)
    w1head = w_tile(0)
    for i in range(4):
        nc.scalar.dma_start(
            w1head[:, :, ts(i, 256)],
            ins["w1"][:, ts(i, 256)].rearrange("(dc p) c -> p dc c", p=P))
    w1_tiles = {4: w1_load(4), 5: w1_load(5)}

    # ---- out-projection + residual -> ao (fp32, token-major) ----
    xn2T = pools["xnqT"].tile([P, DC, SQ], BF16, tag="xnqT", name="xn2T")
    ao_list = []
    for t in range(TQ):
        ao_t = pools["ao"].tile([P, D], F32, tag="ao", name="ao")
        ao_list.append(ao_t)
        xqr_ts = []
        for n in range(2):
            xqr_t = pools["xqr"].tile([P, 512], F32, tag="xqr", name="xqr")
            nc.sync.dma_start(xqr_t[:], xqr[ts(t, P), ts(n, 512)])
            xqr_ts.append(xqr_t)
        ps = pools["score"].tile([P, 1024], F32, tag="score", name="psao")
        for d in range(DC):
            if t == 0 and d == 4:
                emit_normalize(HP - 1)
            lhs = ctxU[:, d, ts(t, P)]
            nc.tensor.matmul(ps[:, 0:512], lhs, wo_sb[:, d, 0:512],
                             start=(d == 0), stop=(d == DC - 1),
                             skip_group_check=True)
            nc.tensor.matmul(ps[:, 512:1024], lhs, wo_sb[:, d, 512:1024],
                             start=(d == 0), stop=(d == DC - 1),
                             skip_group_check=True)
        for n in range(2):
            nc.vector.tensor_tensor(ao_t[:, ts(n, 512)], ps[:, ts(n, 512)],
                                    xqr_ts[n][:], op=ALU.add)
        # LN2 for this tile + spill ao to DRAM for the fc2 residual
        with tc.high_priority():
            xn_t = pools["xn"].tile([P, D], BF16, tag="xn", name="xn")
            ln_tile(ao_t[:], xn_t[:])
            nc.sync.dma_start(xn2T[:, :, ts(t, P)], xn_t[:], transpose=True)
        nc.sync.dma_start(ao_dram[ts(t, P), :], ao_t[:])

    # ---- MLP fc1: h1T feature-major with fused gelu+bias ----
    h1gA = pools["xnT"].tile([P, MC // 2, SQ], BF16, tag="xnTa", name="h1gA")
    h1gB = pools["xnT"].tile([P, MC // 2, SQ], BF16, tag="xnTb", name="h1gB")

    def h1g(m):
        return h1gA[:, m, :] if m < MC // 2 else h1gB[:, m - MC // 2, :]

    # w2 prefetched into the dead knT/vaug regions in 1MB pieces. Each piece's
    # write WARs a dummy read placed inside its address range and tied to fc1
    # progress, spacing the loads so they never hog the DMA engines when the
    # w1 chunk stream needs them.
    w2_tiles = [
        pools["knT"].tile([P, MC, 512], BF16, tag="knT", name="w2n0"),
        pools["vaug"].tile([P, MC, 512], BF16, tag="vaug", name="w2n1"),
    ]
    dummy = pools["const"].tile([1, 16], F32, tag="dummy", name="dummy")
    # dummy source addresses inside each 8KB-per-partition quarter
    knT_probe = [knT[0:1, g, 0, 0:2] for g in range(4)]
    vaug_probe = [vaug[0:1, 4 * g, 0, 0:2] for g in range(4)]

    def w2_piece(half, piece):
        nc.gpsimd.dma_start(
            w2_tiles[half][:, ts(piece, MC // 4), :],
            ins["w2"][ds(piece * (MLP // 4), MLP // 4), ts(half, 512)]
                .rearrange("(mc p) c -> p mc c", p=P))

    w2_gate = {}  # m -> (half, piece)
    for i, m_at in enumerate((1, 3, 5, 7)):
        w2_gate[m_at] = (0, i)
    for i, m_at in enumerate((9, 12, 15, 18)):
        w2_gate[m_at] = (1, i)

    for m in range(MC):
        c = m // CH
        ps = pools["mm512"].tile([P, 512], F32, tag="mm512", name="mm512")
        for d in range(DC):
            lhs = (w1head[:, d, ts(m, P)] if m < 8
                   else w1_tiles[c][:, d, ts(m % CH, P)])
            nc.tensor.matmul(ps[:], lhs, xn2T[:, d, :],
                             start=(d == 0), stop=(d == DC - 1))
        nc.scalar.activation(h1g(m), ps[:], AF.Gelu,
                             bias=bias_m[:, m:m + 1])
        if m in w2_gate:
            half, piece = w2_gate[m]
            probe = (knT_probe if half == 0 else vaug_probe)[piece]
            nc.vector.tensor_tensor(dummy[0:1, ts(half * 4 + piece, 2)],
                                    probe, h1g(m)[0:1, 0:2], op=ALU.add)
            w2_piece(half, piece)
        if m % CH == CH - 1 and 6 <= c + 2 < NCH:
            w1_tiles[c + 2] = w1_load(c + 2)

    # ---- MLP fc2 + bias + residual -> y ----
    for n in range(2):
        w2_n = w2_tiles[n]
        for t in range(TQ):
            ps = pools["mm512"].tile([P, 512], F32, tag="mm512", name="mm512")
            for m in range(MC):
                nc.tensor.matmul(ps[:], h1g(m)[:, ts(t, P)], w2_n[:, m, :],
                                 start=(m == 0), stop=False)
            nc.tensor.matmul(ps[:], ones_tok[:], bias_rhs("b2", n),
                             start=False, stop=True)
            aor = pools["xqr"].tile([P, 512], F32, tag="xqr", name="aor")
            nc.sync.dma_start(aor[:], ao_dram[ts(t, P), ts(n, 512)])
            y_t = pools["qk"].tile([P, 512], F32, tag="qk", name="yout")
            nc.vector.tensor_tensor(y_t[:], ps[:], aor[:], op=ALU.add)
            nc.sync.dma_start(y[ts(t, P), ts(n, 512)], y_t[:])


def build_program(repeat=1, skip_cc=False):
    global SKIP_CC
    SKIP_CC = skip_cc
    nc = bacc.Bacc("TRN2", target_bir_lowering=False, debug=False)
    ins = {}

    def din(name, shape, dt=F32):
        ins[name] = nc.dram_tensor(name, list(shape), dt, kind="ExternalInput").ap()

    din("xq", [SQ, D]); din("xqr", [SQ, D])
    din("wq", [D, D], BF16); din("wk", [D, D], BF16); din("wv", [D, D], BF16)
    din("wo", [D, D], BF16)
    din("w1", [D, MLP], BF16); din("w2", [MLP, D], BF16)
    din("bias_qkv", [1, 3, D], BF16)
    din("bias_m", [P, MC]); din("b2", [1, D], BF16); din("ck", [1, H])
    outs = {"y": nc.dram_tensor("y", [SQ, D], F32, kind="ExternalOutput").ap()}

    with tile.TileContext(nc) as tc:
        with ExitStack() as es:
            pools = {}

            def pool(name, bufs, space="SBUF"):
                pools[name] = es.enter_context(
                    tc.tile_pool(name=name, bufs=bufs, space=space))

            pool("const", 1)
            pool("xnT", 1); pool("xnqT", 1); pool("knT", 1); pool("qnT", 1)
            pool("vaug", 1); pool("xqr", 2); pool("ao", 2); pool("ctxU", 1)
            pool("den", 2); pool("rrow", 1)
            pool("xin", 1); pool("xn", 3); pool("stats", 3)
            pool("qk", 2); pool("w", 1); pool("w1", 2)
            pool("eT", 3)
            pool("dram", 1, space="DRAM")
            pool("mm512", 2, space="PSUM")
            pool("score", 2, space="PSUM")
            pool("ctx", 2, space="PSUM")
            for _ in range(repeat):
                _emit_once(tc, outs, ins, pools)
    nc.compile()
    return nc


def _host_prep(inputs):
    """Host-side slicing + folding. Returns per-core in_maps."""
    f32 = np.float32
    bf16 = ml_dtypes.bfloat16
    x = np.asarray(inputs["x"], f32)
    ln1_g = np.asarray(inputs["ln1_g"], f32); ln1_b = np.asarray(inputs["ln1_b"], f32)
    ln2_g = np.asarray(inputs["ln2_g"], f32); ln2_b = np.asarray(inputs["ln2_b"], f32)
    wq = np.asarray(inputs["wq"], f32); wk = np.asarray(inputs["wk"], f32)
    wv = np.asarray(inputs["wv"], f32); wo = np.asarray(inputs["wo"], f32)
    w1 = np.asarray(inputs["w1"], f32); w2 = np.asarray(inputs["w2"], f32)
    bq = np.asarray(inputs["bq"], f32); bk = np.asarray(inputs["bk"], f32)
    bv = np.asarray(inputs["bv"], f32); bo = np.asarray(inputs["bo"], f32)
    b1 = np.asarray(inputs["b1"], f32); b2 = np.asarray(inputs["b2"], f32)
    ls = np.asarray(inputs["logit_scale"], f32).reshape(H)

    shared = dict(
        wq=(ln1_g[:, None] * wq).astype(bf16),
        wk=(ln1_g[:, None] * wk).astype(bf16),
        wv=(ln1_g[:, None] * wv).astype(bf16),
        wo=wo.astype(bf16),
        w1=(ln2_g[:, None] * w1).astype(bf16),
        w2=w2.astype(bf16),
        bias_qkv=np.stack([ln1_b @ wq + bq, ln1_b @ wk + bk,
                           ln1_b @ wv + bv], axis=0)[None].astype(bf16),
        bias_m=(ln2_b @ w1 + b1).astype(f32).reshape(MC, P).T.copy(),
        b2=b2.astype(bf16).reshape(1, D),
        ck=np.exp(np.minimum(ls, LOG_MAX)).astype(f32).reshape(1, H),
    )
    in_maps = []
    for c in range(N_CORES):
        b = c // 4
        t = c % 4
        sl = slice(t * SQ, (t + 1) * SQ)
        m = dict(shared)
        m["xq"] = np.ascontiguousarray(x[b, sl])
        m["xqr"] = np.ascontiguousarray(x[b, sl] + bo[None, :])
        in_maps.append(m)
    return in_maps


def kernel(**inputs):
    if "main" not in _CACHED_NC:
        _CACHED_NC["main"] = build_program()
    nc = _CACHED_NC["main"]
    in_maps = _host_prep(inputs)
    res = run_bass_kernel_spmd(nc, in_maps, core_ids=list(range(N_CORES)))
    y = np.empty((B, S, D), np.float32)
    for c in range(N_CORES):
        b = c // 4
        t = c % 4
        y[b, t * SQ:(t + 1) * SQ] = res.results[c]["y"]
    return y

